# revision 24
# baseline (speedup 1.0000x reference)
"""Trainium2 Bass kernel for KNN-masked multi-head agent-agent attention.

Problem (per scene): N=1024 agents, D=256 model dim, H=4 heads, K=32 nearest
neighbours by distance. Full pipeline:
    top-K mask from distances -> additive bias (-d/50, -inf outside mask)
    -> MHA (shared in-proj, softmax, out-proj) -> residual + LayerNorm.

Sharding: data-parallel over the batch axis B=8 -> one scene per NeuronCore
(8 cores), no collectives. Each core runs the identical program (SPMD) on its
own scene; the host stacks per-core outputs.

Per-core algorithm. Every engine executes its stream in order, so streams are
kept phase-pure: a stalled op would poison everything emitted after it on the
same engine.
  * DVE runs ONLY the selection until the epilogue: 4 rounds of
    (max8 + match_replace imm=-1e30) on nd=-d mark the exact top-32 multiset
    in-place — match_replace replaces the lowest-index occurrence of each of
    the 8 values per round, which reproduces jax.lax.top_k's index
    tie-breaking exactly for any tie multiplicity. nd conversions are all
    hoisted to the front of the Act stream; distance loads are first on the
    SP DMA queue.
  * Pool builds the bias (negative-offset form, fp16):
    bias = 0.02*nd - 44*[not selected]. Selected entries carry only 0.02*nd
    (full fp16 precision); masked entries sit near -44 where precision is
    irrelevant and exp underflows fp16 to exactly 0. The natural-layout fp16
    bias is transposed by the DMA xbar (dma_start_transpose) into
    per-key-block layout.
  * attention per 256-query chunk, emitted two selection tiles behind so no
    engine waits: S^T = K_h Q_h^T (f32r) + identity-matmul accumulation of
    the transposed fp16 bias, probs = exp(PSUM) in fp16 (one activation per
    4-key-block PSUM group), AV^T in fp16 with a ones-augmented V so the
    softmax denominator falls out of the same matmul. Normalization divides
    by a PE selector-matmul broadcast of the raw denominators (Pool divide,
    no reciprocal -> no DVE). The V in-proj bias is NOT added to V: softmax
    weights sum to 1, so its contribution bv @ Wo^T is folded into the
    output-bias constant instead. Out-proj + residual complete in-chunk.
  * late phase: LayerNorm epilogue (bn_stats on DVE after the last selection,
    scale/shift on Pool) + stores.
"""

import os
import sys
import numpy as np

sys.path.insert(0, "/opt/trn_rl_repo")

import concourse.bass as bass
import concourse.tile as tile
from concourse import mybir
from concourse.masks import make_identity

f32 = mybir.dt.float32
f32r = mybir.dt.float32r
f16 = mybir.dt.float16
Alu = mybir.AluOpType
Act = mybir.ActivationFunctionType

N = 1024
D = 256
H = 4
HD = 64
NT = N // 128          # 8 query/token tiles
KB = N // 128          # 8 key blocks
D_REF = 50.0
LN_EPS = 1e-5
NEG_BIG = -1.0e30
MASK_M = -44.0         # additive mask for non-selected entries (exp -> 0)

MM_DT = f32r


def build_nc(K: int, split_waits: bool = True):
    nc = bass.Bass("TRN2", target_bir_lowering=False, debug=False)

    x_d = nc.dram_tensor("repr1", [N, D], f32, kind="ExternalInput").ap()
    d_d = nc.dram_tensor("distances", [N, N], f32, kind="ExternalInput").ap()
    wi_d = nc.dram_tensor("in_proj_w", [3 * D, D], f32, kind="ExternalInput").ap()
    bi_d = nc.dram_tensor("in_proj_b", [3 * D], f32, kind="ExternalInput").ap()
    wo_d = nc.dram_tensor("out_proj_w", [D, D], f32, kind="ExternalInput").ap()
    bo_d = nc.dram_tensor("out_proj_b", [D], f32, kind="ExternalInput").ap()
    g_d = nc.dram_tensor("ln_gamma", [D], f32, kind="ExternalInput").ap()
    be_d = nc.dram_tensor("ln_beta", [D], f32, kind="ExternalInput").ap()
    out_d = nc.dram_tensor("out", [N, D], f32, kind="ExternalOutput").ap()

    with tile.TileContext(nc) as tc:
        _emit(tc, K, x_d, d_d, wi_d, bi_d, wo_d, bo_d, g_d, be_d, out_d)
    if split_waits:
        _split_waits(nc)
    return nc


def _split_waits(nc, max_waits: int = 1):
    """Walrus codegen rejects instructions carrying more than one sync wait
    (e.g. transpose-matmul LDW structs and HWDGE DMA descriptors), and the
    DMA_DIRECT2D_XPOSE struct carries none at all. Move the excess waits onto
    engine NoOps issued immediately before — the sequencer stalls on those
    first, which is semantically identical."""
    k = 0
    for fn in nc.m.functions:
        for blk in fn.blocks:
            new = []
            for ins in blk.instructions:
                si = ins.sync_info
                mw = 0 if isinstance(ins, mybir.InstDmaTransposeAnt) else max_waits
                if si is not None and si.on_wait and len(si.on_wait) > mw:
                    waits = list(si.on_wait)
                    keep = waits[-mw:] if mw else []
                    for w in (waits[:-mw] if mw else waits):
                        nop = mybir.InstNoOp(
                            name=f"I-wsplit-{k}", engine=ins.engine)
                        nop.sync_info = mybir.SyncInfo(on_wait=[w], on_update=[])
                        new.append(nop)
                        k += 1
                    ins.sync_info = mybir.SyncInfo(
                        on_wait=keep, on_update=list(si.on_update))
                new.append(ins)
            blk.instructions[:] = new


def _bcast_dram_row(nc, dst, src_ap, offset, width):
    """DMA-replicate a [width] DRAM row into all 128 partitions of dst."""
    rep = bass.AP(
        tensor=src_ap.tensor,
        offset=src_ap.offset + offset,
        ap=[[0, 128], [1, width]],
    )
    nc.gpsimd.dma_start(out=dst, in_=rep)


def _emit(tc, K, x_d, d_d, wi_d, bi_d, wo_d, bo_d, g_d, be_d, out_d):
    from contextlib import ExitStack
    nc = tc.nc
    ctx = ExitStack()

    consts = ctx.enter_context(tc.tile_pool(name="consts", bufs=1))
    persist = ctx.enter_context(tc.tile_pool(name="persist", bufs=1))
    dstage = ctx.enter_context(tc.tile_pool(name="dstage", bufs=4))
    dpre = ctx.enter_context(tc.tile_pool(name="dpre", bufs=3))
    ndp = ctx.enter_context(tc.tile_pool(name="ndp", bufs=1))
    selp = ctx.enter_context(tc.tile_pool(name="selp", bufs=3))
    ptp = ctx.enter_context(tc.tile_pool(name="ptp", bufs=3))
    epi = ctx.enter_context(tc.tile_pool(name="epi", bufs=6))
    ps_s = ctx.enter_context(tc.tile_pool(name="ps_s", bufs=2, space="PSUM"))
    ps_av = ctx.enter_context(tc.tile_pool(name="ps_av", bufs=1, space="PSUM"))
    ps_tr = ctx.enter_context(tc.tile_pool(name="ps_tr", bufs=1, space="PSUM"))
    ps_o = ctx.enter_context(tc.tile_pool(name="ps_o", bufs=1, space="PSUM"))
    ps_rb = ctx.enter_context(tc.tile_pool(name="ps_rb", bufs=1, space="PSUM"))

    # ------- distance prefetch (SP queue first) + upfront nd conversion ----
    nds = [ndp.tile([128, N], f32, name=f"nd{i}") for i in range(NT)]
    for i in range(NT):
        drow = dpre.tile([128, N], f32, name="drow", tag="drow")
        nc.sync.dma_start(out=drow, in_=d_d[i * 128:(i + 1) * 128, :])
        nc.scalar.activation(nds[i], drow, Act.Copy, scale=-1.0)  # nd = -d

    # -------- remaining input loads, still on the SP queue -----------------
    wrows, worows, xrows = [], [], []
    for r in range(6):
        t = dstage.tile([128, D], f32, name="wrow", tag="wrow")
        nc.sync.dma_start(out=t, in_=wi_d[r * 128:(r + 1) * 128, :])
        wrows.append(t)
    for r in range(2):
        t = dstage.tile([128, D], f32, name="worow", tag="worow")
        nc.sync.dma_start(out=t, in_=wo_d[r * 128:(r + 1) * 128, :])
        worows.append(t)
    for i in range(NT):
        t = persist.tile([128, D], f32, name=f"xrow{i}")
        nc.sync.dma_start(out=t, in_=x_d[i * 128:(i + 1) * 128, :])
        xrows.append(t)
    bqk = []
    for mb in range(4):
        t = consts.tile([128, 1], f32, name=f"bqk{mb}")
        nc.sync.dma_start(out=t, in_=bi_d[mb * 128:(mb + 1) * 128].rearrange(
            "(p o) -> p o", o=1))
        bqk.append(t)
    bvcol32 = []
    for c in range(2):
        t = consts.tile([128, 1], f32, name=f"bvcol32{c}")
        nc.sync.dma_start(out=t, in_=bi_d[2 * D + c * 128:2 * D + (c + 1) * 128]
                          .rearrange("(p o) -> p o", o=1))
        bvcol32.append(t)

    # ---------------- constants (Pool) ----------------
    ident = consts.tile([128, 128], f32, name="ident")
    make_identity(nc, ident)
    identh = consts.tile([128, 128], f16, name="identh")
    nc.gpsimd.tensor_copy(identh, ident)
    identwarm = ps_tr.tile([128, 128], f32, name="identwarm", tag="wtr")
    nc.tensor.matmul(identwarm, lhsT=ident, rhs=ident, is_transpose=True)

    epsc = consts.tile([128, 1], f32, name="epsc")
    nc.gpsimd.memset(epsc, LN_EPS)
    onesrow = consts.tile([1, 128], f16, name="onesrow")
    nc.gpsimd.memset(onesrow, 1.0)
    bvcol = []
    for c in range(2):
        t = consts.tile([128, 1], f16, name=f"bvcol{c}")
        nc.gpsimd.tensor_copy(t, bvcol32[c])
        bvcol.append(t)

    # head-half selector for the denominator broadcast matmul:
    # sel2[0, p] = [p < 64], sel2[1, p] = [p >= 64]
    sel2 = consts.tile([2, 128], f32, name="sel2")
    iota128 = consts.tile([2, 128], f32, name="iota128")
    nc.gpsimd.iota(iota128, pattern=[[1, 128]], base=0, channel_multiplier=0,
                   allow_small_or_imprecise_dtypes=True)
    nc.gpsimd.tensor_scalar(sel2[0:1, :], iota128[0:1, :], float(HD), None,
                            Alu.is_lt)
    nc.gpsimd.tensor_scalar(sel2[1:2, :], iota128[1:2, :], float(HD), None,
                            Alu.is_ge)

    bo_b = consts.tile([128, D], f32, name="bo_b")
    _bcast_dram_row(nc, bo_b, bo_d, 0, D)
    g_b = consts.tile([128, D], f32, name="g_b")
    _bcast_dram_row(nc, g_b, g_d, 0, D)
    be_b = consts.tile([128, D], f32, name="be_b")
    _bcast_dram_row(nc, be_b, be_d, 0, D)

    # -------- weights / X: PE transposes + Act copies (after the nds) ------
    wt = [persist.tile([128, 3 * D], f32, name=f"wt{c}") for c in range(2)]
    for r in range(6):
        for c in range(2):
            pt = ps_tr.tile([128, 128], f32, name="wtr", tag="wtr")
            nc.tensor.matmul(pt, lhsT=wrows[r][:, c * 128:(c + 1) * 128],
                             rhs=ident, is_transpose=True)
            nc.gpsimd.tensor_copy(wt[c][:, r * 128:(r + 1) * 128].bitcast(f32r), pt)
    # fold the attention scale 1/8 into Wq^T (free cols 0..255 = Q features)
    for c in range(2):
        nc.gpsimd.tensor_scalar_mul(wt[c][:, 0:D].bitcast(f32r), wt[c][:, 0:D], 0.125)
    for mb in range(2):
        nc.gpsimd.tensor_scalar_mul(bqk[mb], bqk[mb], 0.125)

    wot = [persist.tile([128, D], f16, name=f"wot{c}") for c in range(2)]
    for r in range(2):
        for c in range(2):
            pt = ps_tr.tile([128, 128], f32, name="wotr", tag="wtr")
            nc.tensor.matmul(pt, lhsT=worows[r][:, c * 128:(c + 1) * 128],
                             rhs=ident, is_transpose=True)
            nc.gpsimd.tensor_copy(wot[c][:, r * 128:(r + 1) * 128], pt)

    # bo_full = bo + bv @ Wo^T, broadcast into all partitions
    bvwo_ps = ps_rb.tile([1, D], f32, name="bvwo_ps", tag="ps_rb")
    for c in range(2):
        nc.tensor.matmul(bvwo_ps, lhsT=bvcol[c], rhs=wot[c],
                         start=(c == 0), stop=(c == 1))
    bvwo = consts.tile([1, D], f16, name="bvwo")
    nc.scalar.activation(bvwo, bvwo_ps, Act.Copy)
    bvwo_b = ps_rb.tile([128, D], f32, name="bvwo_b", tag="ps_rb")
    nc.tensor.matmul(bvwo_b, lhsT=onesrow, rhs=bvwo)
    bo_full = consts.tile([128, D], f32, name="bo_full")
    nc.gpsimd.tensor_tensor(bo_full, bo_b, bvwo_b, Alu.add)

    xt = [persist.tile([128, N], f32, name=f"xt{c}") for c in range(2)]
    for i in range(NT):
        for c in range(2):
            pt = ps_tr.tile([128, 128], f32, name="xtr", tag="wtr")
            nc.tensor.matmul(pt, lhsT=xrows[i][:, c * 128:(c + 1) * 128],
                             rhs=ident, is_transpose=True)
            nc.gpsimd.tensor_copy(xt[c][:, i * 128:(i + 1) * 128].bitcast(f32r), pt)

    # ---------------- Q^T, K^T, V ----------------
    qkt = [persist.tile([128, N], f32, name=f"qkt{mb}") for mb in range(4)]
    for mb in range(4):
        for qc in range(4):
            ps = ps_o.tile([128, D], f32, name="qk_ps", tag="ps_o")
            for c in range(2):
                nc.tensor.matmul(
                    ps,
                    lhsT=wt[c][:, mb * 128:(mb + 1) * 128].bitcast(MM_DT),
                    rhs=xt[c][:, qc * 256:(qc + 1) * 256].bitcast(MM_DT),
                    start=(c == 0), stop=(c == 1))
            nc.scalar.activation(qkt[mb][:, qc * 256:(qc + 1) * 256].bitcast(f32r),
                                 ps, Act.Identity, bias=bqk[mb])

    # V padded per head, fp16: [128, H, 65]; col 64 of each head slot is the
    # ones column that produces the softmax denominator in the AV matmul.
    # V carries NO in-proj bias — softmax rows sum to 1, so the bias
    # contributes bv @ Wo^T to the output, folded into bo_full below.
    vpad = [persist.tile([128, H, HD + 1], f16, name=f"vpad{kb}") for kb in range(KB)]
    for kb in range(KB):
        nc.gpsimd.memset(vpad[kb][:, :, HD:HD + 1], 1.0)
        ps = ps_o.tile([128, D], f32, name="v_ps", tag="ps_o")
        for c in range(2):
            nc.tensor.matmul(
                ps,
                lhsT=xt[c][:, kb * 128:(kb + 1) * 128].bitcast(MM_DT),
                rhs=wt[c][:, 2 * D:3 * D].bitcast(MM_DT),
                start=(c == 0), stop=(c == 1))
        nc.scalar.activation(
            vpad[kb][:, :, 0:HD],
            ps.rearrange("p (h e) -> p h e", h=H), Act.Copy)


    # ---------------- selection / bias / attention ------------------------
    bias_t = persist.tile([128, KB, N], f16, name="bias_t")
    attnt = [persist.tile([128, N], f16, name=f"attnt{c}") for c in range(2)]
    den2 = [persist.tile([2, N], f32, name=f"den2{c}") for c in range(2)]
    xb = []
    for i in range(NT):
        xb.append(persist.tile([128, D], f32, name=f"xb{i}"))
    xs = []      # residual+attn rows awaiting LayerNorm
    finish = []  # chunks awaiting their Pool finish

    def select_tile(i):
        # DVE: the exact top-32 multiset marking
        nd = nds[i]
        m32 = selp.tile([128, 32], f32, name="m32", tag="m32")
        sc = selp.tile([128, N], f32, name="selsc", tag="selsc")
        nc.vector.max(m32[:, 0:8], nd)
        nc.vector.match_replace(sc, m32[:, 0:8], nd, NEG_BIG)
        nc.vector.max(m32[:, 8:16], sc)
        nc.vector.match_replace(sc, m32[:, 8:16], sc, NEG_BIG)
        nc.vector.max(m32[:, 16:24], sc)
        nc.vector.match_replace(sc, m32[:, 16:24], sc, NEG_BIG)
        nc.vector.max(m32[:, 24:32], sc)
        nc.vector.match_replace(sc, m32[:, 24:32], sc, NEG_BIG)
        # sc == NEG_BIG exactly marks the reference top-32 multiset.

        # Pool: m40 = -44 where NOT selected, 0 where selected
        m40 = selp.tile([128, N], f32, name="m40", tag="m40")
        nc.gpsimd.tensor_scalar(m40, sc, 0.5 * NEG_BIG, MASK_M,
                                Alu.is_gt, Alu.mult)
        # Pool: biasf = 0.02*nd + m40  (fp16 out)
        biasf = selp.tile([128, N], f16, name="biasf", tag="biasf")
        nc.gpsimd.scalar_tensor_tensor(
            out=biasf, in0=nd, scalar=1.0 / D_REF, in1=m40,
            op0=Alu.mult, op1=Alu.add)
        # transpose into bias_t columns i*128..(i+1)*128 (runs on DMA xbar)
        nc.sync.dma_start_transpose(
            out=bias_t[:, :, i * 128:(i + 1) * 128], in_=biasf)

    def attn_chunk(q0, QW):
        # PE + Act + Pool only — no DVE, no stalls for anything not ready.
        qs = slice(q0, q0 + QW)
        for h in range(H):
            qmb, kmb = h // 2, 2 + h // 2
            p0 = (h % 2) * HD
            pt_groups = []
            for g in range(2):      # 2 groups of 4 key blocks
                psf = ps_s.tile([128, 4, 256], f32, name="s_ps", tag="ps_s")
                ps = psf[:, :, 0:QW]
                for j in range(4):
                    kb = 4 * g + j
                    nc.tensor.matmul(
                        ps[:, j, :],
                        lhsT=qkt[kmb][p0:p0 + HD, kb * 128:(kb + 1) * 128].bitcast(MM_DT),
                        rhs=qkt[qmb][p0:p0 + HD, qs].bitcast(MM_DT),
                        start=True, stop=False)
                    nc.tensor.matmul(
                        ps[:, j, :], lhsT=identh, rhs=bias_t[:, kb, qs],
                        start=False, stop=True)
                ptgf = ptp.tile([128, 4, 256], f16, name="pt", tag="pt")
                ptg = ptgf[:, :, 0:QW]
                nc.scalar.activation(ptg, ps, Act.Exp)
                pt_groups.append(ptg)
            av = ps_av.tile([HD + 1, QW], f32, name="av_ps", tag="ps_av")
            for kb in range(KB):
                nc.tensor.matmul(
                    av,
                    lhsT=vpad[kb][:, h, :],
                    rhs=pt_groups[kb // 4][:, kb % 4, :],
                    start=(kb == 0), stop=(kb == KB - 1))
            nc.scalar.activation(
                attnt[h // 2][(h % 2) * HD:(h % 2) * HD + HD, qs],
                av[0:HD, :], Act.Copy)
            nc.scalar.activation(den2[h // 2][h % 2:h % 2 + 1, qs],
                                 av[HD:HD + 1, :], Act.Copy)

        # normalize part 1: broadcast raw denominators via PE + fp16 copy
        rbhs = []
        for c in range(2):
            rbp = ps_rb.tile([128, QW], f32, name="rb_ps", tag="ps_rb")
            nc.tensor.matmul(rbp, lhsT=sel2.bitcast(MM_DT),
                             rhs=den2[c][:, qs].bitcast(MM_DT))
            rbh = epi.tile([128, 256], f16, name="rbh", tag="rbh")
            nc.scalar.activation(rbh[:, 0:QW], rbp, Act.Copy)
            rbhs.append(rbh)
        finish.append((q0, QW, rbhs))

    def chunk_finish():
        # Pool divide + out-proj + residual, emitted ~2 selection pairs after
        # the chunk's main part so the Pool stream never waits.
        q0, QW, rbhs = finish.pop(0)
        qs = slice(q0, q0 + QW)
        for c in range(2):
            nc.gpsimd.tensor_tensor(attnt[c][:, qs], attnt[c][:, qs],
                                    rbhs[c][:, 0:QW], Alu.divide)
        for tb in range(q0 // 128, (q0 + QW) // 128):
            nc.gpsimd.tensor_tensor(xb[tb], xrows[tb], bo_full, Alu.add)
            po = ps_o.tile([128, D], f32, name="o_ps", tag="ps_o")
            for c in range(2):
                nc.tensor.matmul(
                    po,
                    lhsT=attnt[c][:, tb * 128:(tb + 1) * 128],
                    rhs=wot[c],
                    start=(c == 0), stop=(c == 1))
            x = persist.tile([128, D], f32, name=f"x_epi{tb}")
            nc.gpsimd.tensor_tensor(x, po, xb[tb], Alu.add)
            xs.append(x)

    # interleave with lags so every chunk's inputs are ready by the time
    # each in-order engine stream reaches its ops; the last 256 queries run
    # as two 128-query chunks so the post-selection tail is short, and the
    # Pool finish of each chunk trails by ~two selection pairs.
    select_tile(0)
    select_tile(1)
    select_tile(2)
    select_tile(3)
    attn_chunk(0, 256)
    select_tile(4)
    select_tile(5)
    attn_chunk(256, 256)
    chunk_finish()            # chunk 0
    select_tile(6)
    select_tile(7)
    attn_chunk(512, 256)
    chunk_finish()            # chunk 1
    attn_chunk(768, 128)
    chunk_finish()            # chunk 2
    attn_chunk(896, 128)
    chunk_finish()            # chunk 3
    chunk_finish()            # chunk 4

    # ---------------- late phase: LayerNorm epilogue (stage-batched) ------
    lnp = ctx.enter_context(tc.tile_pool(name="lnp", bufs=1))
    sts = [lnp.tile([128, 6], f32, name=f"st{i}") for i in range(NT)]
    mvs = [lnp.tile([128, 2], f32, name=f"mv{i}") for i in range(NT)]
    sds = [lnp.tile([128, 1], f32, name=f"sd{i}") for i in range(NT)]
    rstds = [lnp.tile([128, 1], f32, name=f"rstd{i}") for i in range(NT)]
    xcs = [lnp.tile([128, D], f32, name=f"xc{i}") for i in range(NT)]
    ys = [lnp.tile([128, D], f32, name=f"y{i}") for i in range(NT)]
    for tb in range(NT):
        nc.vector.bn_stats(sts[tb], xs[tb])
        nc.vector.bn_aggr(mvs[tb], sts[tb])
    for tb in range(NT):
        nc.scalar.activation(sds[tb], mvs[tb][:, 1:2], Act.Sqrt, bias=epsc)
    for tb in range(NT):
        nc.vector.reciprocal(rstds[tb], sds[tb])
    for tb in range(NT):
        nc.gpsimd.tensor_scalar(xcs[tb], xs[tb], mvs[tb][:, 0:1], None,
                                Alu.subtract)
    for tb in range(NT):
        nc.vector.scalar_tensor_tensor(
            out=ys[tb], in0=g_b, scalar=rstds[tb], in1=xcs[tb],
            op0=Alu.mult, op1=Alu.mult)
    for tb in range(NT):
        nc.gpsimd.tensor_tensor(ys[tb], ys[tb], be_b, Alu.add)
    for tb in range(NT):
        nc.scalar.dma_start(out=out_d[tb * 128:(tb + 1) * 128, :], in_=ys[tb])

    ctx.close()


_NC_CACHE = {}


def _get_nc(K: int):
    if K not in _NC_CACHE:
        _NC_CACHE[K] = build_nc(K)
    return _NC_CACHE[K]


def kernel(**inputs) -> np.ndarray:
    from concourse.bass_utils import run_bass_kernel_spmd

    K = int(np.asarray(inputs["K"]))
    assert K == 32, f"kernel specialized for K=32, got {K}"
    B = inputs["repr1"].shape[0]
    nc = _get_nc(K)

    shared = {
        "in_proj_w": np.ascontiguousarray(inputs["in_proj_w"], np.float32),
        "in_proj_b": np.ascontiguousarray(inputs["in_proj_b"], np.float32),
        "out_proj_w": np.ascontiguousarray(inputs["out_proj_w"], np.float32),
        "out_proj_b": np.ascontiguousarray(inputs["out_proj_b"], np.float32),
        "ln_gamma": np.ascontiguousarray(inputs["ln_gamma"], np.float32),
        "ln_beta": np.ascontiguousarray(inputs["ln_beta"], np.float32),
    }
    in_maps = []
    for b in range(B):
        m = dict(shared)
        m["repr1"] = np.ascontiguousarray(inputs["repr1"][b], np.float32)
        m["distances"] = np.ascontiguousarray(inputs["distances"][b], np.float32)
        in_maps.append(m)

    res = run_bass_kernel_spmd(nc, in_maps, list(range(B)))
    out = np.stack([np.asarray(res.results[b]["out"]) for b in range(B)])
    return out.astype(np.float32)
